# revision 5
# baseline (speedup 1.0000x reference)
"""Two-layer GCN (scalar-feature factored form) on 8 Trainium2 NeuronCores.

Math (derived from the reference GCNConv stack; features factor out because
x is [N,1] and W1 is [1,35]):
  deg[v]  = indeg_with_self_loops(v)
  dinv    = rsqrt(deg)
  p       = dinv * x                            (pass-1 gather table)
  s1[v]   = dinv[v] * sum_{u in N(v) + v} p[u]  (gather + reduce)
  h2[v]   = sum_k sigmoid(s1[v]*W1[k] + b1[k]) * W2[k]
  q       = dinv * h2                           (pass-2 table, AllGather'd)
  s2[v]   = dinv[v] * sum_{u in N(v) + v} q[u]
  out[v]  = sigmoid(s2[v] + b2)

Sharding: nodes degree-sorted (desc) and dealt round-robin to 8 cores
(load balance + tight per-128-node-block CSR padding). Each core owns
12544 nodes laid out [128 partitions x 98 cols] (in-core index
i = col*128 + part). Gathers read a DRAM table with one extra zero entry
addressed by padding slots.
"""
import os
import numpy as np

N_NODES = 100000
N_PAD = 100352            # 128*784 = 8*12544
N_CORES = 8
PER_CORE = 12544          # 128*98
COLS_OWN = 98
COLS_FULL = 784
PAD_ADDR = N_PAD          # index of the zero entry in both tables

LAST_RESULT = None        # test harness reads exec time from here


def _prep(x, edge_index):
    """CPU-side sharding/layout: permutations + per-core CSR slot indices."""
    x = np.asarray(x, dtype=np.float32).reshape(-1)
    ei = np.asarray(edge_index)
    src = ei[0].astype(np.int64)
    dst = ei[1].astype(np.int64)

    deg = np.bincount(dst, minlength=N_NODES) + 1   # with self-loop
    deg_full = np.ones(N_PAD, np.int64)
    deg_full[:N_NODES] = deg
    x_full = np.zeros(N_PAD, np.float32)
    x_full[:N_NODES] = x

    order = np.argsort(-deg_full, kind="stable")    # rank -> node
    rank = np.empty(N_PAD, np.int64)
    rank[order] = np.arange(N_PAD)

    # node placement: core = rank%8, i = rank//8, part = i%128, col = i//128
    core_of = rank % N_CORES
    i_of = rank // N_CORES
    part_of = i_of % 128
    col_of = i_of // 128

    # pass-1 table layout (row-major [128, 784], full col = 98*core + col):
    addr1 = part_of * COLS_FULL + (COLS_OWN * core_of + col_of)
    # pass-2 table layout (AllGather concat of per-core row-major [128,98]):
    addr2 = PER_CORE * core_of + part_of * COLS_OWN + col_of

    # full-layout inputs [128, 784] matching addr1
    x_in = np.zeros((128, COLS_FULL), np.float32)
    deg_in = np.ones((128, COLS_FULL), np.float32)
    x_in[part_of, COLS_OWN * core_of + col_of] = x_full
    deg_in[part_of, COLS_OWN * core_of + col_of] = deg_full.astype(np.float32)

    r_dst = rank[dst]
    core_e = r_dst % N_CORES
    i_e = r_dst // N_CORES
    a1_src = addr1[src]
    a2_src = addr2[src]

    cores = []
    for c in range(N_CORES):
        sel = core_e == c
        i_sel = i_e[sel]
        a1 = a1_src[sel]
        a2 = a2_src[sel]
        o = np.argsort(i_sel, kind="stable")
        i_sorted = i_sel[o]
        a1 = a1[o]
        a2 = a2[o]

        counts = np.bincount(i_sel, minlength=PER_CORE)      # in-deg per i
        degc = counts + 1                                    # + self slot
        Kb = degc.reshape(COLS_OWN, 128).max(axis=1)         # i = col*128+part
        offs = np.zeros(COLS_OWN + 1, np.int64)
        offs[1:] = np.cumsum(Kb)

        nodes_c = order[np.arange(PER_CORE) * N_CORES + c]
        deg_own = degc.reshape(COLS_OWN, 128).T.astype(np.float32)
        cores.append(dict(
            offs=offs, Kb=Kb, counts=counts, nodes=nodes_c,
            i_sorted=i_sorted, a1=a1, a2=a2, deg_own=deg_own,
        ))

    # uniform (SPMD-shared) block structure: per-block max K over cores
    Kb_u = np.maximum.reduce([c["Kb"] for c in cores])
    offs_u = np.zeros(COLS_OWN + 1, np.int64)
    offs_u[1:] = np.cumsum(Kb_u)
    S = int(offs_u[-1])

    for c in range(N_CORES):
        cc = cores[c]
        idx1 = np.full((128, S), PAD_ADDR, np.int32)
        idx2 = np.full((128, S), PAD_ADDR, np.int32)
        # self slots at k=0 of each node's block range
        p_all = np.arange(PER_CORE) % 128
        b_all = np.arange(PER_CORE) // 128
        idx1[p_all, offs_u[b_all]] = addr1[cc["nodes"]]
        idx2[p_all, offs_u[b_all]] = addr2[cc["nodes"]]
        # in-edges at k = 1 + position within node's run
        counts = cc["counts"]
        starts = np.zeros(PER_CORE, np.int64)
        starts[1:] = np.cumsum(counts)[:-1]
        i_sorted = cc["i_sorted"]
        pos = np.arange(len(i_sorted)) - starts[i_sorted]
        pe = i_sorted % 128
        be = i_sorted // 128
        sc = offs_u[be] + 1 + pos
        idx1[pe, sc] = cc["a1"]
        idx2[pe, sc] = cc["a2"]
        cc["idx1"] = idx1
        cc["idx2"] = idx2

    return dict(order=order, x_in=x_in, deg_in=deg_in, cores=cores,
                offs_u=offs_u, S=S)


def make_inmaps(meta, W1, b1, W2, b2):
    w1b = np.broadcast_to(np.asarray(W1, np.float32).reshape(1, 35),
                          (128, 35)).copy()
    bb1 = np.broadcast_to(np.asarray(b1, np.float32).reshape(1, 35),
                          (128, 35)).copy()
    w2b = np.broadcast_to(np.asarray(W2, np.float32).reshape(1, 35),
                          (128, 35)).copy()
    bb2 = np.full((128, 1), float(np.asarray(b2).reshape(1)[0]), np.float32)
    in_maps = []
    for c in range(N_CORES):
        cc = meta["cores"][c]
        in_maps.append({
            "xf": meta["x_in"], "degf": meta["deg_in"],
            "dego": cc["deg_own"], "idx1": cc["idx1"], "idx2": cc["idx2"],
            "w1": w1b, "bb1": bb1, "w2": w2b, "bb2": bb2,
        })
    return in_maps


def unshard(meta, per_core_out):
    """per_core_out[c]: [PER_CORE] flat DRAM order (part*98+col)."""
    out_full = np.empty(N_PAD, np.float32)
    p_idx = np.arange(PER_CORE) % 128
    c_idx = np.arange(PER_CORE) // 128
    flat = p_idx * COLS_OWN + c_idx
    for c in range(N_CORES):
        nodes_c = meta["cores"][c]["nodes"]
        out_full[nodes_c] = per_core_out[c].reshape(PER_CORE)[flat]
    return out_full[:N_NODES].reshape(N_NODES, 1).astype(np.float32)


def _build_program(meta):
    import concourse.bass as bass
    import concourse.mybir as mybir
    from concourse.bass import IndirectOffsetOnAxis

    f32 = mybir.dt.float32
    i32 = mybir.dt.int32
    AF = mybir.ActivationFunctionType
    OP = mybir.AluOpType

    S = meta["S"]
    offs_u = meta["offs_u"]

    nc = bass.Bass()
    xf_d = nc.dram_tensor("xf", [128, COLS_FULL], f32, kind="ExternalInput")
    degf_d = nc.dram_tensor("degf", [128, COLS_FULL], f32, kind="ExternalInput")
    dego_d = nc.dram_tensor("dego", [128, COLS_OWN], f32, kind="ExternalInput")
    idx1_d = nc.dram_tensor("idx1", [128, S], i32, kind="ExternalInput")
    idx2_d = nc.dram_tensor("idx2", [128, S], i32, kind="ExternalInput")
    w1_d = nc.dram_tensor("w1", [128, 35], f32, kind="ExternalInput")
    bb1_d = nc.dram_tensor("bb1", [128, 35], f32, kind="ExternalInput")
    w2_d = nc.dram_tensor("w2", [128, 35], f32, kind="ExternalInput")
    bb2_d = nc.dram_tensor("bb2", [128, 1], f32, kind="ExternalInput")
    outp_d = nc.dram_tensor("outp", [PER_CORE, 1], f32, kind="ExternalOutput")

    ptab = nc.dram_tensor("ptab", [N_PAD + 1, 1], f32)
    qown = nc.dram_tensor("qown", [PER_CORE, 1], f32)
    qtab = nc.dram_tensor("qtab", [N_PAD + 1, 1], f32, addr_space="Shared")

    from contextlib import ExitStack
    es = ExitStack()
    _n = [0]
    def sb(shape, dt):
        _n[0] += 1
        return es.enter_context(nc.sbuf_tensor(f"sb{_n[0]}", shape, dt))
    sem = lambda name: es.enter_context(nc.semaphore(name))
    xf_sb = sb([128, COLS_FULL], f32); degf_sb = sb([128, COLS_FULL], f32)
    dinvf_sb = sb([128, COLS_FULL], f32); p_sb = sb([128, COLS_FULL], f32)
    dego_sb = sb([128, COLS_OWN], f32); dinvo_sb = sb([128, COLS_OWN], f32)
    idx1_sb = sb([128, S], i32); idx2_sb = sb([128, S], i32)
    val_sb = sb([128, S], f32)
    sacc_sb = sb([128, COLS_OWN], f32); s1_sb = sb([128, COLS_OWN], f32)
    sigA_sb = sb([128, COLS_OWN], f32); sigB_sb = sb([128, COLS_OWN], f32)
    accA_sb = sb([128, COLS_OWN], f32); accB_sb = sb([128, COLS_OWN], f32)
    qown_sb = sb([128, COLS_OWN], f32); s2_sb = sb([128, COLS_OWN], f32)
    out_sb = sb([128, COLS_OWN], f32); zero_sb = sb([1, 1], f32)
    rcpf_sb = sb([128, COLS_FULL], f32); rcpo_sb = sb([128, COLS_OWN], f32)
    w1_sb = sb([128, 35], f32); bb1_sb = sb([128, 35], f32)
    w2_sb = sb([128, 35], f32); bb2_sb = sb([128, 1], f32)
    dsem = sem("dsem"); gs1 = sem("gs1"); gs2 = sem("gs2")
    vs = sem("vs"); as_ = sem("as_"); pwa = sem("pwa"); pwv = sem("pwv")
    ccs = sem("ccs")
    with es:
      with nc.Block() as block:
        ptab_v = ptab[0:N_PAD, 0:1].rearrange("(p c) one -> p (c one)", p=128)
        qown_v = qown[:, 0:1].rearrange("(p c) one -> p (c one)", p=128)
        outp_v = outp_d[:, 0:1].rearrange("(p c) one -> p (c one)", p=128)

        @block.gpsimd
        def _(g):
            d = 0
            for sb, dr in ((xf_sb, xf_d), (degf_sb, degf_d), (dego_sb, dego_d),
                           (idx1_sb, idx1_d), (idx2_sb, idx2_d),
                           (w1_sb, w1_d), (bb1_sb, bb1_d), (w2_sb, w2_d),
                           (bb2_sb, bb2_d)):
                g.dma_start(sb[:], dr[:]).then_inc(dsem, 16)
                d += 16
            g.memset(zero_sb[:], 0.0)
            # publish pass-1 table (+ zero pad entries in both tables)
            g.wait_ge(vs, 3)
            g.dma_start(ptab_v, p_sb[:]).then_inc(dsem, 16); d += 16
            g.dma_start(ptab[N_PAD:N_PAD + 1, 0:1], zero_sb[:]).then_inc(dsem, 16); d += 16
            g.dma_start(qtab[N_PAD:N_PAD + 1, 0:1], zero_sb[:]).then_inc(dsem, 16); d += 16
            g.wait_ge(dsem, d)
            for s in range(S):
                g.indirect_dma_start(
                    out=val_sb[:, s:s + 1], out_offset=None,
                    in_=ptab[:, :],
                    in_offset=IndirectOffsetOnAxis(ap=idx1_sb[:, s:s + 1], axis=0),
                ).then_inc(gs1, 16)
            # publish pass-2 table and gather again
            g.wait_ge(vs, 5)
            g.dma_start(qown_v, qown_sb[:]).then_inc(dsem, 16); d += 16
            g.wait_ge(dsem, d)
            g.collective_compute(
                "AllGather", OP.bypass,
                replica_groups=[list(range(N_CORES))],
                ins=[qown[:, 0:1]],
                outs=[qtab[0:N_PAD, 0:1]],
            ).then_inc(ccs, 1)
            g.wait_ge(ccs, 1)
            for s in range(S):
                g.indirect_dma_start(
                    out=val_sb[:, s:s + 1], out_offset=None,
                    in_=qtab[:, :],
                    in_offset=IndirectOffsetOnAxis(ap=idx2_sb[:, s:s + 1], axis=0),
                ).then_inc(gs2, 16)
            g.wait_ge(as_, 3)
            g.dma_start(outp_v, out_sb[:]).then_inc(dsem, 16); d += 16
            g.wait_ge(dsem, d)

        @block.scalar
        def _(a):
            a.wait_ge(vs, 1)
            a.activation(dinvf_sb[:], rcpf_sb[:], AF.Sqrt).then_inc(as_, 1)
            a.wait_ge(vs, 2)
            a.activation(dinvo_sb[:], rcpo_sb[:], AF.Sqrt).then_inc(as_, 1)
            a.wait_ge(vs, 4)
            for k in range(35):
                buf = sigA_sb if k % 2 == 0 else sigB_sb
                if k >= 2:
                    a.wait_ge(pwv, k - 1)
                a.activation(
                    buf[:], s1_sb[:], AF.Sigmoid,
                    bias=bb1_sb[:, k:k + 1], scale=w1_sb[:, k:k + 1],
                ).then_inc(pwa, 1)
            a.wait_ge(vs, 6)
            a.activation(out_sb[:], s2_sb[:], AF.Sigmoid,
                         bias=bb2_sb[:, 0:1]).then_inc(as_, 1)

        @block.vector
        def _(v):
            v.wait_ge(dsem, 144)   # all 9 input loads
            v.reciprocal(rcpf_sb[:], degf_sb[:]).then_inc(vs, 1)
            v.reciprocal(rcpo_sb[:], dego_sb[:]).then_inc(vs, 1)
            v.wait_ge(as_, 1)
            v.tensor_tensor(out=p_sb[:], in0=dinvf_sb[:], in1=xf_sb[:],
                            op=OP.mult).then_inc(vs, 1)
            v.wait_ge(gs1, 16 * S)
            for b in range(COLS_OWN):
                lo, hi = int(offs_u[b]), int(offs_u[b + 1])
                v.tensor_reduce(out=sacc_sb[:, b:b + 1], in_=val_sb[:, lo:hi],
                                axis=mybir.AxisListType.X, op=OP.add)
            v.wait_ge(as_, 2)
            v.tensor_tensor(out=s1_sb[:], in0=sacc_sb[:], in1=dinvo_sb[:],
                            op=OP.mult).then_inc(vs, 1)
            for k in range(35):
                sig = sigA_sb if k % 2 == 0 else sigB_sb
                v.wait_ge(pwa, k + 1)
                if k == 0:
                    v.tensor_scalar_mul(accA_sb[:], sig[:], w2_sb[:, 0:1]) \
                        .then_inc(pwv, 1)
                else:
                    src_acc = accA_sb if k % 2 == 1 else accB_sb
                    dst_acc = accB_sb if k % 2 == 1 else accA_sb
                    v.scalar_tensor_tensor(
                        out=dst_acc[:], in0=sig[:], scalar=w2_sb[:, k:k + 1],
                        in1=src_acc[:], op0=OP.mult, op1=OP.add) \
                        .then_inc(pwv, 1)
            h2 = accA_sb   # k=34 (even) wrote accA
            v.tensor_tensor(out=qown_sb[:], in0=h2[:], in1=dinvo_sb[:],
                            op=OP.mult).then_inc(vs, 1)
            v.wait_ge(gs2, 16 * S)
            for b in range(COLS_OWN):
                lo, hi = int(offs_u[b]), int(offs_u[b + 1])
                v.tensor_reduce(out=sacc_sb[:, b:b + 1], in_=val_sb[:, lo:hi],
                                axis=mybir.AxisListType.X, op=OP.add)
            v.tensor_tensor(out=s2_sb[:], in0=sacc_sb[:], in1=dinvo_sb[:],
                            op=OP.mult).then_inc(vs, 1)

    return nc


def kernel(x, edge_index, W1, b1, W2, b2):
    global LAST_RESULT
    from concourse.bass_utils import run_bass_kernel_spmd

    meta = _prep(x, edge_index)
    nc = _build_program(meta)
    in_maps = make_inmaps(meta, W1, b1, W2, b2)

    trace = os.environ.get("BASS_KERNEL_TRACE", "0") == "1"
    res = run_bass_kernel_spmd(nc, in_maps, list(range(N_CORES)), trace=trace)
    LAST_RESULT = res
    return unshard(meta, [res.results[c]["outp"] for c in range(N_CORES)])


# revision 7
# speedup vs baseline: 1.8722x; 1.8722x over previous
"""Two-layer GCN (scalar-feature factored form) on 8 Trainium2 NeuronCores.

Math (features factor out because x is [N,1] and W1 is [1,35]):
  deg[v]  = indeg_with_self_loops(v);  dinv = rsqrt(deg)
  p       = dinv * x                            (pass-1 gather table)
  s1[v]   = dinv[v] * sum_{u in N(v)+v} p[u]
  h2[v]   = sum_k sigmoid(s1[v]*W1[k] + b1[k]) * W2[k]
  q       = dinv * h2                           (pass-2 table, AllGather'd)
  s2[v]   = dinv[v] * sum_{u in N(v)+v} q[u]
  out[v]  = sigmoid(s2[v] + b2)

Sharding: nodes degree-sorted (desc), dealt round-robin to 8 cores; each
core owns 12544 nodes indexed i = 0..12543 (degree-sorted). Nodes are
grouped into 256 bands of 49; band r has uniform slot width K_r =
max degree in band (over all cores, so the program is SPMD-shared).
Band r's slots live in one SBUF partition row (part = r%128, segment
r//128); a single indirect-DMA instruction gathers up to 2048 table
entries into that row (descriptor-per-element; indices consumed
column-major over a 128-partition wrap). Padding slots address a zero
table entry. Per-band free-dim reduces are pipelined behind the gather
stream.
"""
import os
import numpy as np

N_NODES = 100000
N_PAD = 100352            # 128*784 = 8*12544
N_CORES = 8
PER_CORE = 12544          # 256 bands * 49
COLS_OWN = 98             # own tiles [128, 98]: (part, half*49 + j)
COLS_FULL = 784
NB = 256                  # bands per core
BN = 49                   # nodes per band
PAD_ADDR = N_PAD          # zero entry index in both tables
CHUNK = 2048              # max descriptors per indirect instruction

LAST_RESULT = None


def _prep(x, edge_index):
    x = np.asarray(x, dtype=np.float32).reshape(-1)
    ei = np.asarray(edge_index)
    src = ei[0].astype(np.int64)
    dst = ei[1].astype(np.int64)

    deg = np.bincount(dst, minlength=N_NODES) + 1
    deg_full = np.ones(N_PAD, np.int64)
    deg_full[:N_NODES] = deg
    x_full = np.zeros(N_PAD, np.float32)
    x_full[:N_NODES] = x

    order = np.argsort(-deg_full, kind="stable")
    rank = np.empty(N_PAD, np.int64)
    rank[order] = np.arange(N_PAD)

    core_of = rank % N_CORES
    i_of = rank // N_CORES
    # own-tile placement [128, 98]: part = (i//49)%128, col = (i//6272)*49 + i%49
    part_of = (i_of // BN) % 128
    col_of = (i_of // (BN * 128)) * BN + (i_of % BN)

    addr1 = part_of * COLS_FULL + (COLS_OWN * core_of + col_of)
    addr2 = PER_CORE * core_of + i_of          # pass-2 table is i-order flat

    x_in = np.zeros((128, COLS_FULL), np.float32)
    deg_in = np.ones((128, COLS_FULL), np.float32)
    x_in[part_of, COLS_OWN * core_of + col_of] = x_full
    deg_in[part_of, COLS_OWN * core_of + col_of] = deg_full.astype(np.float32)

    r_dst = rank[dst]
    core_e = r_dst % N_CORES
    i_e_all = r_dst // N_CORES
    a1_src = addr1[src]
    a2_src = addr2[src]

    # per-core degree (in i order) -> shared band widths K_r
    counts_all = []
    for c in range(N_CORES):
        counts_all.append(np.bincount(i_e_all[core_e == c], minlength=PER_CORE))
    degc_all = [cnt + 1 for cnt in counts_all]
    Kr = np.maximum.reduce([d.reshape(NB, BN).max(axis=1) for d in degc_all])
    Kr = np.repeat(Kr.reshape(NB // 32, 32).max(axis=1), 32)  # 32-band groups
    Fr = BN * Kr                                   # used slots per band
    Fpad = ((Fr + 127) // 128) * 128               # wrap-aligned slots
    Wr = Fpad // 128
    Woff = np.zeros(NB + 1, np.int64)
    Woff[1:] = np.cumsum(Wr)
    W = int(Woff[-1])
    fof = Woff * 128                               # flat slot offsets per band
    VCAP = int(Fpad.max())

    # per-band instruction chunks (each <= CHUNK descs, multiple of 128)
    chunks = []           # list of (band, off, ln)
    for r in range(NB):
        off = 0
        while off < Fpad[r]:
            ln = min(CHUNK, int(Fpad[r]) - off)
            chunks.append((r, off, ln))
            off += ln
    cum_instr = np.zeros(NB, np.int64)    # instrs completed once band r done
    seen = 0
    for r in range(NB):
        seen += sum(1 for (b, _, _) in chunks if b == r)
        cum_instr[r] = seen

    cores = []
    for c in range(N_CORES):
        sel = core_e == c
        i_sel = i_e_all[sel]
        a1 = a1_src[sel]
        a2 = a2_src[sel]
        o = np.argsort(i_sel, kind="stable")
        i_sorted = i_sel[o]
        a1 = a1[o]
        a2 = a2[o]
        counts = counts_all[c]
        starts = np.zeros(PER_CORE, np.int64)
        starts[1:] = np.cumsum(counts)[:-1]
        pos = np.arange(len(i_sorted)) - starts[i_sorted]

        band_e = i_sorted // BN
        j_e = i_sorted % BN
        d_edge = fof[band_e] + j_e * Kr[band_e] + 1 + pos
        iarr = np.arange(PER_CORE)
        band_i = iarr // BN
        d_self = fof[band_i] + (iarr % BN) * Kr[band_i]

        nodes_c = order[iarr * N_CORES + c]

        flat1 = np.full(128 * W, PAD_ADDR, np.int64)
        flat2 = np.full(128 * W, PAD_ADDR, np.int64)
        flat1[d_self] = addr1[nodes_c]
        flat2[d_self] = addr2[nodes_c]
        flat1[d_edge] = a1
        flat2[d_edge] = a2

        idx1w = np.empty((128, W), np.int32)
        idx2w = np.empty((128, W), np.int32)
        for r in range(NB):
            seg1 = flat1[fof[r]:fof[r] + Fpad[r]].reshape(Wr[r], 128).T
            seg2 = flat2[fof[r]:fof[r] + Fpad[r]].reshape(Wr[r], 128).T
            idx1w[:, Woff[r]:Woff[r + 1]] = seg1
            idx2w[:, Woff[r]:Woff[r + 1]] = seg2

        degco = degc_all[c]
        deg_own = np.ones((128, COLS_OWN), np.float32)
        deg_own[part_of[nodes_c], col_of[nodes_c]] = degco.astype(np.float32)

        cores.append(dict(nodes=nodes_c, idx1=idx1w, idx2=idx2w,
                          deg_own=deg_own))

    return dict(order=order, x_in=x_in, deg_in=deg_in, cores=cores,
                Kr=Kr, Fpad=Fpad, Woff=Woff, W=W, VCAP=VCAP,
                chunks=chunks, cum_instr=cum_instr)


def make_inmaps(meta, W1, b1, W2, b2):
    w1b = np.broadcast_to(np.asarray(W1, np.float32).reshape(1, 35),
                          (128, 35)).copy()
    bb1 = np.broadcast_to(np.asarray(b1, np.float32).reshape(1, 35),
                          (128, 35)).copy()
    w2b = np.broadcast_to(np.asarray(W2, np.float32).reshape(1, 35),
                          (128, 35)).copy()
    bb2 = np.full((128, 1), float(np.asarray(b2).reshape(1)[0]), np.float32)
    in_maps = []
    for c in range(N_CORES):
        cc = meta["cores"][c]
        in_maps.append({
            "xf": meta["x_in"], "degf": meta["deg_in"],
            "dego": cc["deg_own"], "idx1": cc["idx1"], "idx2": cc["idx2"],
            "w1": w1b, "bb1": bb1, "w2": w2b, "bb2": bb2,
        })
    return in_maps


def unshard(meta, per_core_out):
    """per_core_out[c]: [PER_CORE] in i order."""
    out_full = np.empty(N_PAD, np.float32)
    for c in range(N_CORES):
        out_full[meta["cores"][c]["nodes"]] = per_core_out[c].reshape(PER_CORE)
    return out_full[:N_NODES].reshape(N_NODES, 1).astype(np.float32)


def _build_program(meta):
    import concourse.bass as bass
    import concourse.mybir as mybir
    from concourse.bass import IndirectOffsetOnAxis
    from contextlib import ExitStack

    f32 = mybir.dt.float32
    i32 = mybir.dt.int32
    AF = mybir.ActivationFunctionType
    OP = mybir.AluOpType

    Kr = meta["Kr"]; Woff = meta["Woff"]; W = meta["W"]
    VCAP = meta["VCAP"]; chunks = meta["chunks"]; cum = meta["cum_instr"]

    nc = bass.Bass()
    xf_d = nc.dram_tensor("xf", [128, COLS_FULL], f32, kind="ExternalInput")
    degf_d = nc.dram_tensor("degf", [128, COLS_FULL], f32, kind="ExternalInput")
    dego_d = nc.dram_tensor("dego", [128, COLS_OWN], f32, kind="ExternalInput")
    idx1_d = nc.dram_tensor("idx1", [128, W], i32, kind="ExternalInput")
    idx2_d = nc.dram_tensor("idx2", [128, W], i32, kind="ExternalInput")
    w1_d = nc.dram_tensor("w1", [128, 35], f32, kind="ExternalInput")
    bb1_d = nc.dram_tensor("bb1", [128, 35], f32, kind="ExternalInput")
    w2_d = nc.dram_tensor("w2", [128, 35], f32, kind="ExternalInput")
    bb2_d = nc.dram_tensor("bb2", [128, 1], f32, kind="ExternalInput")
    outp_d = nc.dram_tensor("outp", [PER_CORE, 1], f32, kind="ExternalOutput")

    ptab = nc.dram_tensor("ptab", [N_PAD + 1, 1], f32)
    qown = nc.dram_tensor("qown", [PER_CORE, 1], f32)
    qtab = nc.dram_tensor("qtab", [N_PAD + 1, 1], f32, addr_space="Shared")

    es = ExitStack()
    _n = [0]
    def sb(shape, dt):
        _n[0] += 1
        return es.enter_context(nc.sbuf_tensor(f"sb{_n[0]}", shape, dt))
    sem = lambda name: es.enter_context(nc.semaphore(name))

    xf_sb = sb([128, COLS_FULL], f32); degf_sb = sb([128, COLS_FULL], f32)
    dinvf_sb = sb([128, COLS_FULL], f32); p_sb = sb([128, COLS_FULL], f32)
    rcpf_sb = sb([128, COLS_FULL], f32); rcpo_sb = sb([128, COLS_OWN], f32)
    dego_sb = sb([128, COLS_OWN], f32); dinvo_sb = sb([128, COLS_OWN], f32)
    idx1_sb = sb([128, W], i32); idx2_sb = sb([128, W], i32)
    val_sb = sb([128, 2 * VCAP], f32)
    sacc_sb = sb([128, COLS_OWN], f32); s1_sb = sb([128, COLS_OWN], f32)
    sigA_sb = sb([128, COLS_OWN], f32); sigB_sb = sb([128, COLS_OWN], f32)
    accA_sb = sb([128, COLS_OWN], f32); accB_sb = sb([128, COLS_OWN], f32)
    qown_sb = sb([128, COLS_OWN], f32); s2_sb = sb([128, COLS_OWN], f32)
    out_sb = sb([128, COLS_OWN], f32); zero_sb = sb([1, 1], f32)
    w1_sb = sb([128, 35], f32); bb1_sb = sb([128, 35], f32)
    w2_sb = sb([128, 35], f32); bb2_sb = sb([128, 1], f32)
    dsem = sem("dsem"); gs1 = sem("gs1"); gs2 = sem("gs2")
    vs = sem("vs"); as_ = sem("as_"); pwa = sem("pwa"); pwv = sem("pwv")
    ccs = sem("ccs")

    with es:
      with nc.Block() as block:
        ptab_v = ptab[0:N_PAD, 0:1].rearrange("(p c) one -> p (c one)", p=128)
        # i-order flat <-> SBUF [128, 98]: i = (part + 128*half)*49 + j
        qown_v = qown[:, 0:1].rearrange("(h p j) one -> p h (j one)",
                                        h=2, p=128, j=BN)
        outp_v = outp_d[:, 0:1].rearrange("(h p j) one -> p h (j one)",
                                          h=2, p=128, j=BN)
        qown_in = qown_sb[:].rearrange("p (h j) -> p h j", h=2)
        out_in = out_sb[:].rearrange("p (h j) -> p h j", h=2)

        def gathers(g, table, idx_sb, gsem):
            for (r, off, ln) in chunks:
                part, seg = r % 128, r // 128
                dst = val_sb[part:part + 1,
                             seg * VCAP + off : seg * VCAP + off + ln]
                g.indirect_dma_start(
                    out=dst.rearrange("p (f one) -> p f one", one=1),
                    out_offset=None,
                    in_=table[:, :],
                    in_offset=IndirectOffsetOnAxis(
                        ap=idx_sb[:, int(Woff[r]) + off // 128 :
                                  int(Woff[r]) + (off + ln) // 128],
                        axis=0),
                ).then_inc(gsem, 16)

        def reduces(v, gsem):
            for grp in range(NB // 32):
                r0 = grp * 32
                p0, seg = r0 % 128, r0 // 128
                K = int(Kr[r0])
                v.wait_ge(gsem, 16 * int(cum[r0 + 31]))
                src = val_sb[p0:p0 + 32, seg * VCAP : seg * VCAP + BN * K]
                v.tensor_reduce(
                    out=sacc_sb[p0:p0 + 32, seg * BN : seg * BN + BN],
                    in_=src.rearrange("p (j k) -> p j k", k=K),
                    axis=mybir.AxisListType.X, op=OP.add)

        @block.gpsimd
        def _(g):
            d = 0
            for sbuf, dr in ((xf_sb, xf_d), (degf_sb, degf_d), (dego_sb, dego_d),
                             (idx1_sb, idx1_d), (idx2_sb, idx2_d),
                             (w1_sb, w1_d), (bb1_sb, bb1_d), (w2_sb, w2_d),
                             (bb2_sb, bb2_d)):
                g.dma_start(sbuf[:], dr[:]).then_inc(dsem, 16)
                d += 16
            g.memset(zero_sb[:], 0.0)
            g.wait_ge(vs, 3)
            g.dma_start(ptab_v, p_sb[:]).then_inc(dsem, 16); d += 16
            g.dma_start(ptab[N_PAD:N_PAD + 1, 0:1], zero_sb[:]).then_inc(dsem, 16); d += 16
            g.dma_start(qtab[N_PAD:N_PAD + 1, 0:1], zero_sb[:]).then_inc(dsem, 16); d += 16
            g.wait_ge(dsem, d)
            gathers(g, ptab, idx1_sb, gs1)
            g.wait_ge(vs, 5)
            g.dma_start(qown_v, qown_in).then_inc(dsem, 16); d += 16
            g.wait_ge(dsem, d)
            g.collective_compute(
                "AllGather", OP.bypass,
                replica_groups=[list(range(N_CORES))],
                ins=[qown[:, 0:1]],
                outs=[qtab[0:N_PAD, 0:1]],
            ).then_inc(ccs, 1)
            g.wait_ge(ccs, 1)
            gathers(g, qtab, idx2_sb, gs2)
            g.wait_ge(as_, 3)
            g.dma_start(outp_v, out_in).then_inc(dsem, 16); d += 16
            g.wait_ge(dsem, d)

        @block.scalar
        def _(a):
            a.wait_ge(vs, 1)
            a.activation(dinvf_sb[:], rcpf_sb[:], AF.Sqrt).then_inc(as_, 1)
            a.wait_ge(vs, 2)
            a.activation(dinvo_sb[:], rcpo_sb[:], AF.Sqrt).then_inc(as_, 1)
            a.wait_ge(vs, 4)
            for k in range(35):
                buf = sigA_sb if k % 2 == 0 else sigB_sb
                if k >= 2:
                    a.wait_ge(pwv, k - 1)
                a.activation(
                    buf[:], s1_sb[:], AF.Sigmoid,
                    bias=bb1_sb[:, k:k + 1], scale=w1_sb[:, k:k + 1],
                ).then_inc(pwa, 1)
            a.wait_ge(vs, 6)
            a.activation(out_sb[:], s2_sb[:], AF.Sigmoid,
                         bias=bb2_sb[:, 0:1]).then_inc(as_, 1)

        @block.vector
        def _(v):
            v.wait_ge(dsem, 144)
            v.reciprocal(rcpf_sb[:], degf_sb[:]).then_inc(vs, 1)
            v.reciprocal(rcpo_sb[:], dego_sb[:]).then_inc(vs, 1)
            v.wait_ge(as_, 1)
            v.tensor_tensor(out=p_sb[:], in0=dinvf_sb[:], in1=xf_sb[:],
                            op=OP.mult).then_inc(vs, 1)
            reduces(v, gs1)
            v.wait_ge(as_, 2)
            v.tensor_tensor(out=s1_sb[:], in0=sacc_sb[:], in1=dinvo_sb[:],
                            op=OP.mult).then_inc(vs, 1)
            for k in range(35):
                sig = sigA_sb if k % 2 == 0 else sigB_sb
                v.wait_ge(pwa, k + 1)
                if k == 0:
                    v.tensor_scalar_mul(accA_sb[:], sig[:], w2_sb[:, 0:1]) \
                        .then_inc(pwv, 1)
                else:
                    src_acc = accA_sb if k % 2 == 1 else accB_sb
                    dst_acc = accB_sb if k % 2 == 1 else accA_sb
                    v.scalar_tensor_tensor(
                        out=dst_acc[:], in0=sig[:], scalar=w2_sb[:, k:k + 1],
                        in1=src_acc[:], op0=OP.mult, op1=OP.add) \
                        .then_inc(pwv, 1)
            v.tensor_tensor(out=qown_sb[:], in0=accA_sb[:], in1=dinvo_sb[:],
                            op=OP.mult).then_inc(vs, 1)
            reduces(v, gs2)
            v.tensor_tensor(out=s2_sb[:], in0=sacc_sb[:], in1=dinvo_sb[:],
                            op=OP.mult).then_inc(vs, 1)

    return nc


def kernel(x, edge_index, W1, b1, W2, b2):
    global LAST_RESULT
    from concourse.bass_utils import run_bass_kernel_spmd

    meta = _prep(x, edge_index)
    nc = _build_program(meta)
    in_maps = make_inmaps(meta, W1, b1, W2, b2)

    trace = os.environ.get("BASS_KERNEL_TRACE", "0") == "1"
    res = run_bass_kernel_spmd(nc, in_maps, list(range(N_CORES)), trace=trace)
    LAST_RESULT = res
    return unshard(meta, [res.results[c]["outp"] for c in range(N_CORES)])


# revision 8
# speedup vs baseline: 1.8799x; 1.0041x over previous
"""Two-layer GCN (scalar-feature factored form) on 8 Trainium2 NeuronCores.

Math (features factor out because x is [N,1] and W1 is [1,35]):
  deg[v]  = indeg_with_self_loops(v);  dinv = rsqrt(deg)
  p       = dinv * x                            (pass-1 gather table)
  s1[v]   = dinv[v] * sum_{u in N(v)+v} p[u]
  h2[v]   = sum_k sigmoid(s1[v]*W1[k] + b1[k]) * W2[k]
  q       = dinv * h2                           (pass-2 table, AllGather'd)
  s2[v]   = dinv[v] * sum_{u in N(v)+v} q[u]
  out[v]  = sigmoid(s2[v] + b2)

Sharding: nodes degree-sorted (desc), dealt round-robin to 8 cores; each
core owns 12544 nodes indexed i = 0..12543 (degree-sorted). Nodes are
grouped into 256 bands of 49; band r has uniform slot width K_r =
max degree in band (over all cores, so the program is SPMD-shared).
Band r's slots live in one SBUF partition row (part = r%128, segment
r//128); a single indirect-DMA instruction gathers up to 2048 table
entries into that row (descriptor-per-element; indices consumed
column-major over a 128-partition wrap). Padding slots address a zero
table entry. Per-band free-dim reduces are pipelined behind the gather
stream.
"""
import os
import numpy as np

N_NODES = 100000
N_PAD = 100352            # 128*784 = 8*12544
N_CORES = 8
PER_CORE = 12544          # 256 bands * 49
COLS_OWN = 98             # own tiles [128, 98]: (part, half*49 + j)
COLS_FULL = 784
NB = 256                  # bands per core
BN = 49                   # nodes per band
PAD_ADDR = N_PAD          # zero entry index in both tables
CHUNK = 2048              # max descriptors per indirect instruction

LAST_RESULT = None


def _prep(x, edge_index):
    x = np.asarray(x, dtype=np.float32).reshape(-1)
    ei = np.asarray(edge_index)
    src = ei[0].astype(np.int64)
    dst = ei[1].astype(np.int64)

    deg = np.bincount(dst, minlength=N_NODES) + 1
    deg_full = np.ones(N_PAD, np.int64)
    deg_full[:N_NODES] = deg
    x_full = np.zeros(N_PAD, np.float32)
    x_full[:N_NODES] = x

    order = np.argsort(-deg_full, kind="stable")
    rank = np.empty(N_PAD, np.int64)
    rank[order] = np.arange(N_PAD)

    core_of = rank % N_CORES
    i_of = rank // N_CORES
    # own-tile placement [128, 98]: part = (i//49)%128, col = (i//6272)*49 + i%49
    part_of = (i_of // BN) % 128
    col_of = (i_of // (BN * 128)) * BN + (i_of % BN)

    addr1 = part_of * COLS_FULL + (COLS_OWN * core_of + col_of)
    addr2 = PER_CORE * core_of + i_of          # pass-2 table is i-order flat

    x_in = np.zeros((128, COLS_FULL), np.float32)
    deg_in = np.ones((128, COLS_FULL), np.float32)
    x_in[part_of, COLS_OWN * core_of + col_of] = x_full
    deg_in[part_of, COLS_OWN * core_of + col_of] = deg_full.astype(np.float32)

    r_dst = rank[dst]
    core_e = r_dst % N_CORES
    i_e_all = r_dst // N_CORES
    a1_src = addr1[src]
    a2_src = addr2[src]

    # per-core degree (in i order) -> shared band widths K_r
    counts_all = []
    for c in range(N_CORES):
        counts_all.append(np.bincount(i_e_all[core_e == c], minlength=PER_CORE))
    degc_all = [cnt + 1 for cnt in counts_all]
    Kr = np.maximum.reduce([d.reshape(NB, BN).max(axis=1) for d in degc_all])
    Kr = np.repeat(Kr.reshape(NB // 32, 32).max(axis=1), 32)  # 32-band groups
    Fr = BN * Kr                                   # used slots per band
    Fpad = ((Fr + 127) // 128) * 128               # wrap-aligned slots
    Wr = Fpad // 128
    Woff = np.zeros(NB + 1, np.int64)
    Woff[1:] = np.cumsum(Wr)
    W = int(Woff[-1])
    fof = Woff * 128                               # flat slot offsets per band
    VCAP = int(Fpad.max())

    # per-band instruction chunks (each <= CHUNK descs, multiple of 128)
    chunks = []           # list of (band, off, ln)
    for r in range(NB):
        off = 0
        while off < Fpad[r]:
            ln = min(CHUNK, int(Fpad[r]) - off)
            chunks.append((r, off, ln))
            off += ln
    cum_instr = np.zeros(NB, np.int64)    # instrs completed once band r done
    seen = 0
    for r in range(NB):
        seen += sum(1 for (b, _, _) in chunks if b == r)
        cum_instr[r] = seen

    cores = []
    for c in range(N_CORES):
        sel = core_e == c
        i_sel = i_e_all[sel]
        a1 = a1_src[sel]
        a2 = a2_src[sel]
        o = np.argsort(i_sel, kind="stable")
        i_sorted = i_sel[o]
        a1 = a1[o]
        a2 = a2[o]
        counts = counts_all[c]
        starts = np.zeros(PER_CORE, np.int64)
        starts[1:] = np.cumsum(counts)[:-1]
        pos = np.arange(len(i_sorted)) - starts[i_sorted]

        band_e = i_sorted // BN
        j_e = i_sorted % BN
        d_edge = fof[band_e] + j_e * Kr[band_e] + 1 + pos
        iarr = np.arange(PER_CORE)
        band_i = iarr // BN
        d_self = fof[band_i] + (iarr % BN) * Kr[band_i]

        nodes_c = order[iarr * N_CORES + c]

        flat1 = np.full(128 * W, PAD_ADDR, np.int64)
        flat2 = np.full(128 * W, PAD_ADDR, np.int64)
        flat1[d_self] = addr1[nodes_c]
        flat2[d_self] = addr2[nodes_c]
        flat1[d_edge] = a1
        flat2[d_edge] = a2

        idx1w = np.empty((128, W), np.int32)
        idx2w = np.empty((128, W), np.int32)
        for r in range(NB):
            seg1 = flat1[fof[r]:fof[r] + Fpad[r]].reshape(Wr[r], 128).T
            seg2 = flat2[fof[r]:fof[r] + Fpad[r]].reshape(Wr[r], 128).T
            idx1w[:, Woff[r]:Woff[r + 1]] = seg1
            idx2w[:, Woff[r]:Woff[r + 1]] = seg2

        degco = degc_all[c]
        deg_own = np.ones((128, COLS_OWN), np.float32)
        deg_own[part_of[nodes_c], col_of[nodes_c]] = degco.astype(np.float32)

        cores.append(dict(nodes=nodes_c, idx1=idx1w, idx2=idx2w,
                          deg_own=deg_own))

    return dict(order=order, x_in=x_in, deg_in=deg_in, cores=cores,
                Kr=Kr, Fpad=Fpad, Woff=Woff, W=W, VCAP=VCAP,
                chunks=chunks, cum_instr=cum_instr)


def make_inmaps(meta, W1, b1, W2, b2):
    w1b = np.broadcast_to(np.asarray(W1, np.float32).reshape(1, 35),
                          (128, 35)).copy()
    bb1 = np.broadcast_to(np.asarray(b1, np.float32).reshape(1, 35),
                          (128, 35)).copy()
    w2b = np.broadcast_to(np.asarray(W2, np.float32).reshape(1, 35),
                          (128, 35)).copy()
    bb2 = np.full((128, 1), float(np.asarray(b2).reshape(1)[0]), np.float32)
    in_maps = []
    for c in range(N_CORES):
        cc = meta["cores"][c]
        in_maps.append({
            "xf": meta["x_in"], "degf": meta["deg_in"],
            "dego": cc["deg_own"], "idx1": cc["idx1"], "idx2": cc["idx2"],
            "w1": w1b, "bb1": bb1, "w2": w2b, "bb2": bb2,
        })
    return in_maps


def unshard(meta, per_core_out):
    """per_core_out[c]: [PER_CORE] in i order."""
    out_full = np.empty(N_PAD, np.float32)
    for c in range(N_CORES):
        out_full[meta["cores"][c]["nodes"]] = per_core_out[c].reshape(PER_CORE)
    return out_full[:N_NODES].reshape(N_NODES, 1).astype(np.float32)


def _build_program(meta):
    import concourse.bass as bass
    import concourse.mybir as mybir
    from concourse.bass import IndirectOffsetOnAxis
    from contextlib import ExitStack

    f32 = mybir.dt.float32
    i32 = mybir.dt.int32
    AF = mybir.ActivationFunctionType
    OP = mybir.AluOpType

    Kr = meta["Kr"]; Woff = meta["Woff"]; W = meta["W"]
    VCAP = meta["VCAP"]; chunks = meta["chunks"]; cum = meta["cum_instr"]

    nc = bass.Bass(num_swdge_queues=4)
    xf_d = nc.dram_tensor("xf", [128, COLS_FULL], f32, kind="ExternalInput")
    degf_d = nc.dram_tensor("degf", [128, COLS_FULL], f32, kind="ExternalInput")
    dego_d = nc.dram_tensor("dego", [128, COLS_OWN], f32, kind="ExternalInput")
    idx1_d = nc.dram_tensor("idx1", [128, W], i32, kind="ExternalInput")
    idx2_d = nc.dram_tensor("idx2", [128, W], i32, kind="ExternalInput")
    w1_d = nc.dram_tensor("w1", [128, 35], f32, kind="ExternalInput")
    bb1_d = nc.dram_tensor("bb1", [128, 35], f32, kind="ExternalInput")
    w2_d = nc.dram_tensor("w2", [128, 35], f32, kind="ExternalInput")
    bb2_d = nc.dram_tensor("bb2", [128, 1], f32, kind="ExternalInput")
    outp_d = nc.dram_tensor("outp", [PER_CORE, 1], f32, kind="ExternalOutput")

    ptab = nc.dram_tensor("ptab", [N_PAD + 1, 1], f32)
    qown = nc.dram_tensor("qown", [PER_CORE, 1], f32)
    qtab = nc.dram_tensor("qtab", [N_PAD + 1, 1], f32, addr_space="Shared")

    es = ExitStack()
    _n = [0]
    def sb(shape, dt):
        _n[0] += 1
        return es.enter_context(nc.sbuf_tensor(f"sb{_n[0]}", shape, dt))
    sem = lambda name: es.enter_context(nc.semaphore(name))

    xf_sb = sb([128, COLS_FULL], f32); degf_sb = sb([128, COLS_FULL], f32)
    dinvf_sb = sb([128, COLS_FULL], f32); p_sb = sb([128, COLS_FULL], f32)
    rcpf_sb = sb([128, COLS_FULL], f32); rcpo_sb = sb([128, COLS_OWN], f32)
    dego_sb = sb([128, COLS_OWN], f32); dinvo_sb = sb([128, COLS_OWN], f32)
    idx1_sb = sb([128, W], i32); idx2_sb = sb([128, W], i32)
    val_sb = sb([128, 2 * VCAP], f32)
    sacc_sb = sb([128, COLS_OWN], f32); s1_sb = sb([128, COLS_OWN], f32)
    sigA_sb = sb([128, COLS_OWN], f32); sigB_sb = sb([128, COLS_OWN], f32)
    accA_sb = sb([128, COLS_OWN], f32); accB_sb = sb([128, COLS_OWN], f32)
    qown_sb = sb([128, COLS_OWN], f32); s2_sb = sb([128, COLS_OWN], f32)
    out_sb = sb([128, COLS_OWN], f32); zero_sb = sb([1, 1], f32)
    w1_sb = sb([128, 35], f32); bb1_sb = sb([128, 35], f32)
    w2_sb = sb([128, 35], f32); bb2_sb = sb([128, 1], f32)
    dsem = sem("dsem"); gs1 = sem("gs1"); gs2 = sem("gs2")
    vs = sem("vs"); as_ = sem("as_"); pwa = sem("pwa"); pwv = sem("pwv")
    ccs = sem("ccs")

    with es:
      with nc.Block() as block:
        ptab_v = ptab[0:N_PAD, 0:1].rearrange("(p c) one -> p (c one)", p=128)
        # i-order flat <-> SBUF [128, 98]: i = (part + 128*half)*49 + j
        qown_v = qown[:, 0:1].rearrange("(h p j) one -> p h (j one)",
                                        h=2, p=128, j=BN)
        outp_v = outp_d[:, 0:1].rearrange("(h p j) one -> p h (j one)",
                                          h=2, p=128, j=BN)
        qown_in = qown_sb[:].rearrange("p (h j) -> p h j", h=2)
        out_in = out_sb[:].rearrange("p (h j) -> p h j", h=2)

        def gathers(g, table, idx_sb, gsem):
            for n, (r, off, ln) in enumerate(chunks):
                part, seg = r % 128, r // 128
                dst = val_sb[part:part + 1,
                             seg * VCAP + off : seg * VCAP + off + ln]
                bi = g.indirect_dma_start(
                    out=dst.rearrange("p (f one) -> p f one", one=1),
                    out_offset=None,
                    in_=table[:, :],
                    in_offset=IndirectOffsetOnAxis(
                        ap=idx_sb[:, int(Woff[r]) + off // 128 :
                                  int(Woff[r]) + (off + ln) // 128],
                        axis=0),
                )
                qn = n % 4
                bi.ins.queue = f"qPoolDynamic{qn or ''}"
                bi.then_inc(gsem, 16)

        def reduces(v, gsem):
            for grp in range(NB // 32):
                r0 = grp * 32
                p0, seg = r0 % 128, r0 // 128
                K = int(Kr[r0])
                v.wait_ge(gsem, 16 * int(cum[r0 + 31]))
                src = val_sb[p0:p0 + 32, seg * VCAP : seg * VCAP + BN * K]
                v.tensor_reduce(
                    out=sacc_sb[p0:p0 + 32, seg * BN : seg * BN + BN],
                    in_=src.rearrange("p (j k) -> p j k", k=K),
                    axis=mybir.AxisListType.X, op=OP.add)

        @block.gpsimd
        def _(g):
            d = 0
            for sbuf, dr in ((xf_sb, xf_d), (degf_sb, degf_d), (dego_sb, dego_d),
                             (idx1_sb, idx1_d), (idx2_sb, idx2_d),
                             (w1_sb, w1_d), (bb1_sb, bb1_d), (w2_sb, w2_d),
                             (bb2_sb, bb2_d)):
                g.dma_start(sbuf[:], dr[:]).then_inc(dsem, 16)
                d += 16
            g.memset(zero_sb[:], 0.0)
            g.wait_ge(vs, 3)
            g.dma_start(ptab_v, p_sb[:]).then_inc(dsem, 16); d += 16
            g.dma_start(ptab[N_PAD:N_PAD + 1, 0:1], zero_sb[:]).then_inc(dsem, 16); d += 16
            g.dma_start(qtab[N_PAD:N_PAD + 1, 0:1], zero_sb[:]).then_inc(dsem, 16); d += 16
            g.wait_ge(dsem, d)
            gathers(g, ptab, idx1_sb, gs1)
            g.wait_ge(vs, 5)
            g.dma_start(qown_v, qown_in).then_inc(dsem, 16); d += 16
            g.wait_ge(dsem, d)
            g.collective_compute(
                "AllGather", OP.bypass,
                replica_groups=[list(range(N_CORES))],
                ins=[qown[:, 0:1]],
                outs=[qtab[0:N_PAD, 0:1]],
            ).then_inc(ccs, 1)
            g.wait_ge(ccs, 1)
            gathers(g, qtab, idx2_sb, gs2)
            g.wait_ge(as_, 3)
            g.dma_start(outp_v, out_in).then_inc(dsem, 16); d += 16
            g.wait_ge(dsem, d)

        @block.scalar
        def _(a):
            a.wait_ge(vs, 1)
            a.activation(dinvf_sb[:], rcpf_sb[:], AF.Sqrt).then_inc(as_, 1)
            a.wait_ge(vs, 2)
            a.activation(dinvo_sb[:], rcpo_sb[:], AF.Sqrt).then_inc(as_, 1)
            a.wait_ge(vs, 4)
            for k in range(35):
                buf = sigA_sb if k % 2 == 0 else sigB_sb
                if k >= 2:
                    a.wait_ge(pwv, k - 1)
                a.activation(
                    buf[:], s1_sb[:], AF.Sigmoid,
                    bias=bb1_sb[:, k:k + 1], scale=w1_sb[:, k:k + 1],
                ).then_inc(pwa, 1)
            a.wait_ge(vs, 6)
            a.activation(out_sb[:], s2_sb[:], AF.Sigmoid,
                         bias=bb2_sb[:, 0:1]).then_inc(as_, 1)

        @block.vector
        def _(v):
            v.wait_ge(dsem, 144)
            v.reciprocal(rcpf_sb[:], degf_sb[:]).then_inc(vs, 1)
            v.reciprocal(rcpo_sb[:], dego_sb[:]).then_inc(vs, 1)
            v.wait_ge(as_, 1)
            v.tensor_tensor(out=p_sb[:], in0=dinvf_sb[:], in1=xf_sb[:],
                            op=OP.mult).then_inc(vs, 1)
            reduces(v, gs1)
            v.wait_ge(as_, 2)
            v.tensor_tensor(out=s1_sb[:], in0=sacc_sb[:], in1=dinvo_sb[:],
                            op=OP.mult).then_inc(vs, 1)
            for k in range(35):
                sig = sigA_sb if k % 2 == 0 else sigB_sb
                v.wait_ge(pwa, k + 1)
                if k == 0:
                    v.tensor_scalar_mul(accA_sb[:], sig[:], w2_sb[:, 0:1]) \
                        .then_inc(pwv, 1)
                else:
                    src_acc = accA_sb if k % 2 == 1 else accB_sb
                    dst_acc = accB_sb if k % 2 == 1 else accA_sb
                    v.scalar_tensor_tensor(
                        out=dst_acc[:], in0=sig[:], scalar=w2_sb[:, k:k + 1],
                        in1=src_acc[:], op0=OP.mult, op1=OP.add) \
                        .then_inc(pwv, 1)
            v.tensor_tensor(out=qown_sb[:], in0=accA_sb[:], in1=dinvo_sb[:],
                            op=OP.mult).then_inc(vs, 1)
            reduces(v, gs2)
            v.tensor_tensor(out=s2_sb[:], in0=sacc_sb[:], in1=dinvo_sb[:],
                            op=OP.mult).then_inc(vs, 1)

    return nc


def kernel(x, edge_index, W1, b1, W2, b2):
    global LAST_RESULT
    from concourse.bass_utils import run_bass_kernel_spmd

    meta = _prep(x, edge_index)
    nc = _build_program(meta)
    in_maps = make_inmaps(meta, W1, b1, W2, b2)

    trace = os.environ.get("BASS_KERNEL_TRACE", "0") == "1"
    res = run_bass_kernel_spmd(nc, in_maps, list(range(N_CORES)), trace=trace)
    LAST_RESULT = res
    return unshard(meta, [res.results[c]["outp"] for c in range(N_CORES)])


# revision 12
# speedup vs baseline: 2.0332x; 1.0816x over previous
"""Two-layer GCN (scalar-feature factored form) on 8 Trainium2 NeuronCores.

Math (features factor out because x is [N,1] and W1 is [1,35]):
  deg[v]  = indeg_with_self_loops(v);  dinv = rsqrt(deg)
  p       = dinv * x                            (pass-1 gather table)
  s1[v]   = dinv[v] * sum_{u in N(v)+v} p[u]
  h2[v]   = sum_k sigmoid(s1[v]*W1[k] + b1[k]) * W2[k]
  q       = dinv * h2                           (pass-2 table, AllGather'd)
  s2[v]   = dinv[v] * sum_{u in N(v)+v} q[u]
  out[v]  = sigmoid(s2[v] + b2)

Sharding: nodes degree-sorted (desc), dealt round-robin to 8 cores; each
core owns 12544 nodes indexed i = 0..12543 (degree-sorted). Nodes are
grouped into 256 bands of 49; band r has uniform slot width K_r =
max degree in band (over all cores, so the program is SPMD-shared).
Band r's slots live in one SBUF partition row (part = r%128, segment
r//128); a single indirect-DMA instruction gathers up to 2048 table
entries into that row (descriptor-per-element; indices consumed
column-major over a 128-partition wrap). Padding slots address a zero
table entry. Per-band free-dim reduces are pipelined behind the gather
stream.
"""
import os
import numpy as np

N_NODES = 100000
N_PAD = 100352            # 128*784 = 8*12544
N_CORES = 8
PER_CORE = 12544          # 256 bands * 49
COLS_OWN = 98             # own tiles [128, 98]: (part, half*49 + j)
COLS_FULL = 784
NB = 256                  # bands per core
BN = 49                   # nodes per band
PAD_ADDR = N_PAD          # zero entry index in both tables
CHUNK = 2048              # max descriptors per indirect instruction

LAST_RESULT = None


def _prep(x, edge_index):
    x = np.asarray(x, dtype=np.float32).reshape(-1)
    ei = np.asarray(edge_index)
    src = ei[0].astype(np.int64)
    dst = ei[1].astype(np.int64)

    deg = np.bincount(dst, minlength=N_NODES) + 1
    deg_full = np.ones(N_PAD, np.int64)
    deg_full[:N_NODES] = deg
    x_full = np.zeros(N_PAD, np.float32)
    x_full[:N_NODES] = x

    order = np.argsort(-deg_full, kind="stable")
    rank = np.empty(N_PAD, np.int64)
    rank[order] = np.arange(N_PAD)

    core_of = rank % N_CORES
    i_of = rank // N_CORES
    # own-tile placement [128, 98]: part = (i//49)%128, col = (i//6272)*49 + i%49
    part_of = (i_of // BN) % 128
    col_of = (i_of // (BN * 128)) * BN + (i_of % BN)

    addr1 = part_of * COLS_FULL + (COLS_OWN * core_of + col_of)
    addr2 = PER_CORE * core_of + i_of          # pass-2 table is i-order flat

    x_in = np.zeros((128, COLS_FULL), np.float32)
    deg_in = np.ones((128, COLS_FULL), np.float32)
    x_in[part_of, COLS_OWN * core_of + col_of] = x_full
    deg_in[part_of, COLS_OWN * core_of + col_of] = deg_full.astype(np.float32)

    r_dst = rank[dst]
    core_e = r_dst % N_CORES
    i_e_all = r_dst // N_CORES
    a1_src = addr1[src]
    a2_src = addr2[src]

    # per-core degree (in i order) -> shared band widths K_r
    counts_all = []
    for c in range(N_CORES):
        counts_all.append(np.bincount(i_e_all[core_e == c], minlength=PER_CORE))
    degc_all = [cnt + 1 for cnt in counts_all]
    Kr = np.maximum.reduce([d.reshape(NB, BN).max(axis=1) for d in counts_all])
    Kr = np.repeat(Kr.reshape(NB // 32, 32).max(axis=1), 32)  # 32-band groups
    Fr = BN * Kr                                   # used slots per band
    Fpad = ((Fr + 127) // 128) * 128               # wrap-aligned slots
    Wr = Fpad // 128
    Woff = np.zeros(NB + 1, np.int64)
    Woff[1:] = np.cumsum(Wr)
    W = int(Woff[-1])
    fof = Woff * 128                               # flat slot offsets per band
    VCAP = int(Fpad.max())

    # per-band instruction chunks (each <= CHUNK descs, multiple of 128)
    chunks = []           # list of (band, off, ln)
    for r in range(NB):
        off = 0
        while off < Fpad[r]:
            ln = min(CHUNK, int(Fpad[r]) - off)
            chunks.append((r, off, ln))
            off += ln
    cum_instr = np.zeros(NB, np.int64)    # instrs completed once band r done
    seen = 0
    for r in range(NB):
        seen += sum(1 for (b, _, _) in chunks if b == r)
        cum_instr[r] = seen

    cores = []
    for c in range(N_CORES):
        sel = core_e == c
        i_sel = i_e_all[sel]
        a1 = a1_src[sel]
        a2 = a2_src[sel]
        o = np.argsort(i_sel, kind="stable")
        i_sorted = i_sel[o]
        a1 = a1[o]
        a2 = a2[o]
        counts = counts_all[c]
        starts = np.zeros(PER_CORE, np.int64)
        starts[1:] = np.cumsum(counts)[:-1]
        pos = np.arange(len(i_sorted)) - starts[i_sorted]

        band_e = i_sorted // BN
        j_e = i_sorted % BN
        d_edge = fof[band_e] + j_e * Kr[band_e] + pos
        iarr = np.arange(PER_CORE)

        nodes_c = order[iarr * N_CORES + c]

        flat1 = np.full(128 * W, PAD_ADDR, np.int64)
        flat2 = np.full(128 * W, PAD_ADDR, np.int64)
        flat1[d_edge] = a1
        flat2[d_edge] = a2

        idx1w = np.empty((128, W), np.int32)
        idx2w = np.empty((128, W), np.int32)
        for r in range(NB):
            seg1 = flat1[fof[r]:fof[r] + Fpad[r]].reshape(Wr[r], 128).T
            seg2 = flat2[fof[r]:fof[r] + Fpad[r]].reshape(Wr[r], 128).T
            idx1w[:, Woff[r]:Woff[r + 1]] = seg1
            idx2w[:, Woff[r]:Woff[r + 1]] = seg2

        degco = degc_all[c]
        deg_own = np.ones((128, COLS_OWN), np.float32)
        deg_own[part_of[nodes_c], col_of[nodes_c]] = degco.astype(np.float32)
        x_own = np.zeros((128, COLS_OWN), np.float32)
        x_own[part_of[nodes_c], col_of[nodes_c]] = x_full[nodes_c]

        cores.append(dict(nodes=nodes_c, idx1=idx1w, idx2=idx2w,
                          deg_own=deg_own, x_own=x_own))

    return dict(order=order, x_in=x_in, deg_in=deg_in, cores=cores,
                Kr=Kr, Fpad=Fpad, Woff=Woff, W=W, VCAP=VCAP,
                chunks=chunks, cum_instr=cum_instr)


def make_inmaps(meta, W1, b1, W2, b2):
    w1b = np.broadcast_to(np.asarray(W1, np.float32).reshape(1, 35),
                          (128, 35)).copy()
    bb1 = np.broadcast_to(np.asarray(b1, np.float32).reshape(1, 35),
                          (128, 35)).copy()
    w2b = np.broadcast_to(np.asarray(W2, np.float32).reshape(1, 35),
                          (128, 35)).copy()
    bb2 = np.full((128, 1), float(np.asarray(b2).reshape(1)[0]), np.float32)
    in_maps = []
    for c in range(N_CORES):
        cc = meta["cores"][c]
        in_maps.append({
            "xf": meta["x_in"], "degf": meta["deg_in"],
            "dego": cc["deg_own"], "xo": cc["x_own"],
            "idx1": cc["idx1"], "idx2": cc["idx2"],
            "w1": w1b, "bb1": bb1, "w2": w2b, "bb2": bb2,
        })
    return in_maps


def unshard(meta, per_core_out):
    """per_core_out[c]: [PER_CORE] in i order."""
    out_full = np.empty(N_PAD, np.float32)
    for c in range(N_CORES):
        out_full[meta["cores"][c]["nodes"]] = per_core_out[c].reshape(PER_CORE)
    return out_full[:N_NODES].reshape(N_NODES, 1).astype(np.float32)


def _build_program(meta):
    import concourse.bass as bass
    import concourse.mybir as mybir
    from concourse.bass import IndirectOffsetOnAxis
    from contextlib import ExitStack

    f32 = mybir.dt.float32
    i32 = mybir.dt.int32
    AF = mybir.ActivationFunctionType
    OP = mybir.AluOpType

    Kr = meta["Kr"]; Woff = meta["Woff"]; W = meta["W"]
    VCAP = meta["VCAP"]; chunks = meta["chunks"]; cum = meta["cum_instr"]

    nc = bass.Bass(num_swdge_queues=4)
    xf_d = nc.dram_tensor("xf", [128, COLS_FULL], f32, kind="ExternalInput")
    degf_d = nc.dram_tensor("degf", [128, COLS_FULL], f32, kind="ExternalInput")
    dego_d = nc.dram_tensor("dego", [128, COLS_OWN], f32, kind="ExternalInput")
    xo_d = nc.dram_tensor("xo", [128, COLS_OWN], f32, kind="ExternalInput")
    idx1_d = nc.dram_tensor("idx1", [128, W], i32, kind="ExternalInput")
    idx2_d = nc.dram_tensor("idx2", [128, W], i32, kind="ExternalInput")
    w1_d = nc.dram_tensor("w1", [128, 35], f32, kind="ExternalInput")
    bb1_d = nc.dram_tensor("bb1", [128, 35], f32, kind="ExternalInput")
    w2_d = nc.dram_tensor("w2", [128, 35], f32, kind="ExternalInput")
    bb2_d = nc.dram_tensor("bb2", [128, 1], f32, kind="ExternalInput")
    outp_d = nc.dram_tensor("outp", [PER_CORE, 1], f32, kind="ExternalOutput")

    ptab = nc.dram_tensor("ptab", [N_PAD + 1, 1], f32)
    qown = nc.dram_tensor("qown", [PER_CORE, 1], f32)
    qtab = nc.dram_tensor("qtab", [N_PAD + 1, 1], f32, addr_space="Shared")

    es = ExitStack()
    _n = [0]
    def sb(shape, dt):
        _n[0] += 1
        return es.enter_context(nc.sbuf_tensor(f"sb{_n[0]}", shape, dt))
    sem = lambda name: es.enter_context(nc.semaphore(name))

    xf_sb = sb([128, COLS_FULL], f32); degf_sb = sb([128, COLS_FULL], f32)
    dinvf_sb = sb([128, COLS_FULL], f32); p_sb = sb([128, COLS_FULL], f32)
    rcpf_sb = sb([128, COLS_FULL], f32); rcpo_sb = sb([128, COLS_OWN], f32)
    dego_sb = sb([128, COLS_OWN], f32); dinvo_sb = sb([128, COLS_OWN], f32)
    xo_sb = sb([128, COLS_OWN], f32); pown_sb = sb([128, COLS_OWN], f32)
    idx1_sb = sb([128, W], i32); idx2_sb = sb([128, W], i32)
    val_sb = sb([128, 2 * VCAP], f32)
    sacc_sb = sb([128, COLS_OWN], f32); s1_sb = sb([128, COLS_OWN], f32)
    sigA_sb = sb([128, COLS_OWN], f32); sigB_sb = sb([128, COLS_OWN], f32)
    accA_sb = sb([128, COLS_OWN], f32); accB_sb = sb([128, COLS_OWN], f32)
    qown_sb = sb([128, COLS_OWN], f32); s2_sb = sb([128, COLS_OWN], f32)
    out_sb = sb([128, COLS_OWN], f32); zero_sb = sb([1, 1], f32)
    w1_sb = sb([128, 35], f32); bb1_sb = sb([128, 35], f32)
    w2_sb = sb([128, 35], f32); bb2_sb = sb([128, 1], f32)
    dsem = sem("dsem")
    gs1 = [sem(f"gs1_{q}") for q in range(4)]
    gs2 = [sem(f"gs2_{q}") for q in range(4)]
    vs = sem("vs"); as_ = sem("as_"); pwa = sem("pwa"); pwv = sem("pwv")
    ccs = sem("ccs")

    with es:
      with nc.Block() as block:
        ptab_v = ptab[0:N_PAD, 0:1].rearrange("(p c) one -> p (c one)", p=128)
        # i-order flat <-> SBUF [128, 98]: i = (part + 128*half)*49 + j
        qown_v = qown[:, 0:1].rearrange("(h p j) one -> p h (j one)",
                                        h=2, p=128, j=BN)
        outp_v = outp_d[:, 0:1].rearrange("(h p j) one -> p h (j one)",
                                          h=2, p=128, j=BN)
        qown_in = qown_sb[:].rearrange("p (h j) -> p h j", h=2)
        out_in = out_sb[:].rearrange("p (h j) -> p h j", h=2)

        def gathers(g, table, idx_sb, gsem):
            for n, (r, off, ln) in enumerate(chunks):
                part, seg = r % 128, r // 128
                dst = val_sb[part:part + 1,
                             seg * VCAP + off : seg * VCAP + off + ln]
                bi = g.indirect_dma_start(
                    out=dst.rearrange("p (f one) -> p f one", one=1),
                    out_offset=None,
                    in_=table[:, :],
                    in_offset=IndirectOffsetOnAxis(
                        ap=idx_sb[:, int(Woff[r]) + off // 128 :
                                  int(Woff[r]) + (off + ln) // 128],
                        axis=0),
                )
                qn = n % 4
                bi.ins.queue = f"qPoolDynamic{qn or ''}"
                bi.then_inc(gsem[qn], 16)

        def reduces(v, gsem):
            for grp in range(NB // 32):
                r0 = grp * 32
                p0, seg = r0 % 128, r0 // 128
                K = int(Kr[r0])
                if K == 0:
                    continue
                Cg = int(cum[r0 + 31])
                for q in range(4):
                    nq = (Cg + 3 - q) // 4
                    if nq > 0:
                        v.wait_ge(gsem[q], 16 * nq)
                src = val_sb[p0:p0 + 32, seg * VCAP : seg * VCAP + BN * K]
                v.tensor_reduce(
                    out=sacc_sb[p0:p0 + 32, seg * BN : seg * BN + BN],
                    in_=src.rearrange("p (j k) -> p j k", k=K),
                    axis=mybir.AxisListType.X, op=OP.add)

        @block.gpsimd
        def _(g):
            d = 0
            for sbuf, dr in ((xf_sb, xf_d), (degf_sb, degf_d), (dego_sb, dego_d),
                             (xo_sb, xo_d),
                             (idx1_sb, idx1_d), (idx2_sb, idx2_d),
                             (w1_sb, w1_d), (bb1_sb, bb1_d), (w2_sb, w2_d),
                             (bb2_sb, bb2_d)):
                g.dma_start(sbuf[:], dr[:]).then_inc(dsem, 16)
                d += 16
            g.memset(zero_sb[:], 0.0)
            g.wait_ge(vs, 3)
            g.dma_start(ptab_v, p_sb[:]).then_inc(dsem, 16); d += 16
            g.dma_start(ptab[N_PAD:N_PAD + 1, 0:1], zero_sb[:]).then_inc(dsem, 16); d += 16
            g.dma_start(qtab[N_PAD:N_PAD + 1, 0:1], zero_sb[:]).then_inc(dsem, 16); d += 16
            g.wait_ge(dsem, d)
            gathers(g, ptab, idx1_sb, gs1)
            g.wait_ge(vs, 5)
            g.dma_start(qown_v, qown_in).then_inc(dsem, 16); d += 16
            g.wait_ge(dsem, d)
            g.collective_compute(
                "AllGather", OP.bypass,
                replica_groups=[list(range(N_CORES))],
                ins=[qown[:, 0:1]],
                outs=[qtab[0:N_PAD, 0:1]],
            ).then_inc(ccs, 1)
            g.wait_ge(ccs, 1)
            gathers(g, qtab, idx2_sb, gs2)
            g.wait_ge(as_, 3)
            g.dma_start(outp_v, out_in).then_inc(dsem, 16); d += 16
            g.wait_ge(dsem, d)

        @block.scalar
        def _(a):
            a.wait_ge(vs, 1)
            a.activation(dinvf_sb[:], rcpf_sb[:], AF.Sqrt).then_inc(as_, 1)
            a.wait_ge(vs, 2)
            a.activation(dinvo_sb[:], rcpo_sb[:], AF.Sqrt).then_inc(as_, 1)
            a.wait_ge(vs, 4)
            for k in range(35):
                buf = sigA_sb if k % 2 == 0 else sigB_sb
                if k >= 2:
                    a.wait_ge(pwv, k - 1)
                a.activation(
                    buf[:], s1_sb[:], AF.Sigmoid,
                    bias=bb1_sb[:, k:k + 1], scale=w1_sb[:, k:k + 1],
                ).then_inc(pwa, 1)
            a.wait_ge(vs, 6)
            a.activation(out_sb[:], s2_sb[:], AF.Sigmoid,
                         bias=bb2_sb[:, 0:1]).then_inc(as_, 1)

        @block.vector
        def _(v):
            v.wait_ge(dsem, 160)
            v.reciprocal(rcpf_sb[:], degf_sb[:]).then_inc(vs, 1)
            v.reciprocal(rcpo_sb[:], dego_sb[:]).then_inc(vs, 1)
            v.wait_ge(as_, 1)
            v.tensor_tensor(out=p_sb[:], in0=dinvf_sb[:], in1=xf_sb[:],
                            op=OP.mult).then_inc(vs, 1)
            v.memset(sacc_sb[:], 0.0)
            reduces(v, gs1)
            v.wait_ge(as_, 2)
            v.tensor_tensor(out=pown_sb[:], in0=dinvo_sb[:], in1=xo_sb[:],
                            op=OP.mult)
            v.tensor_tensor(out=accB_sb[:], in0=sacc_sb[:], in1=pown_sb[:],
                            op=OP.add)
            v.tensor_tensor(out=s1_sb[:], in0=accB_sb[:], in1=dinvo_sb[:],
                            op=OP.mult).then_inc(vs, 1)
            for k in range(35):
                sig = sigA_sb if k % 2 == 0 else sigB_sb
                v.wait_ge(pwa, k + 1)
                if k == 0:
                    v.tensor_scalar_mul(accA_sb[:], sig[:], w2_sb[:, 0:1]) \
                        .then_inc(pwv, 1)
                else:
                    src_acc = accA_sb if k % 2 == 1 else accB_sb
                    dst_acc = accB_sb if k % 2 == 1 else accA_sb
                    v.scalar_tensor_tensor(
                        out=dst_acc[:], in0=sig[:], scalar=w2_sb[:, k:k + 1],
                        in1=src_acc[:], op0=OP.mult, op1=OP.add) \
                        .then_inc(pwv, 1)
            v.tensor_tensor(out=qown_sb[:], in0=accA_sb[:], in1=dinvo_sb[:],
                            op=OP.mult).then_inc(vs, 1)
            v.memset(sacc_sb[:], 0.0)
            reduces(v, gs2)
            v.tensor_tensor(out=accB_sb[:], in0=sacc_sb[:], in1=qown_sb[:],
                            op=OP.add)
            v.tensor_tensor(out=s2_sb[:], in0=accB_sb[:], in1=dinvo_sb[:],
                            op=OP.mult).then_inc(vs, 1)

    return nc


def kernel(x, edge_index, W1, b1, W2, b2):
    global LAST_RESULT
    from concourse.bass_utils import run_bass_kernel_spmd

    meta = _prep(x, edge_index)
    nc = _build_program(meta)
    in_maps = make_inmaps(meta, W1, b1, W2, b2)

    trace = os.environ.get("BASS_KERNEL_TRACE", "0") == "1"
    res = run_bass_kernel_spmd(nc, in_maps, list(range(N_CORES)), trace=trace)
    LAST_RESULT = res
    return unshard(meta, [res.results[c]["outp"] for c in range(N_CORES)])
